# revision 10
# baseline (speedup 1.0000x reference)
"""GroupWiseLinear Trainium2 kernel.

out[b, c] = dot(W[0, c, :], x[b, group_of[c], :]) + bias[0, c], then a final
class-permutation gather, for two independent branches (co / cl).

Sharding: 8 cores = 2 branches x 4 class-shards.  Shard boundaries are chosen
at group boundaries so no group's x is loaded by two cores.  Each core's
ragged class range is cut into "pieces" (one group each, <= 512 classes); the
piece widths of all 8 cores are rank-matched into a single static width
ENVELOPE so every core runs the same instruction stream (SPMD) on different
data:

  - xm: [128, S, 256]    per-slot x^T (one 64KB tile per piece, H-major)
  - wt: [128, 4, CW]     W^T packed to the envelope layout (pad = garbage)
  - o:  [64, CW]         bf16 output, envelope layout (pad ignored on host)

Device work per slot: 4 K-chunk matmuls (x stationary [128,64], W moving
[128, w_env]) accumulating into a PSUM bank shared by several slots; each
bank is then copied (f32->bf16, two engine-parallel halves) to SBUF and
DMA'd out.  Bias and the final class permutation are applied on host.
"""

import heapq

import ml_dtypes
import numpy as np

import concourse.bacc as bacc
import concourse.tile as tile
from concourse import mybir
from concourse.bass_utils import run_bass_kernel_spmd

B = 64          # batch
H = 512         # hidden
NCLS = 4096     # classes per branch
KC = H // 128   # contraction chunks
NQ = 4          # class-shards per branch
BANK = 512      # psum bank width (f32 cols)

_cache = {}


# ----------------------------------------------------------------- planning

def _segments(go):
    """group_of (sorted) -> list of (group, class_start, width)."""
    go = np.asarray(go).astype(np.int64)
    segs = []
    i = 0
    n = len(go)
    while i < n:
        j = i
        while j < n and go[j] == go[i]:
            j += 1
        segs.append((int(go[i]), i, j - i))
        i = j
    return segs


def _core_pieces(segs, S):
    """Split a core's segments into exactly S pieces (halve the largest),
    returning them sorted by descending width.  None if > S segments."""
    if len(segs) > S:
        return None
    h = [(-w, g, cs, w) for (g, cs, w) in segs]
    heapq.heapify(h)
    n = len(h)
    while n < S:
        _, g, cs, w = heapq.heappop(h)
        a = w // 2
        b = w - a
        if a == 0:  # cannot split further; put back and stop
            heapq.heappush(h, (-w, g, cs, w))
            break
        heapq.heappush(h, (-a, g, cs, a))
        heapq.heappush(h, (-b, g, cs + a, b))
        n += 1
    return sorted(((w, g, cs) for (_, g, cs, w) in h), reverse=True)


def _branch_cores(segs, cuts, S):
    """cuts: 3 group-index boundaries -> 4 cores' piece lists."""
    bounds = [0] + list(cuts) + [len(segs)]
    out = []
    for a, b in zip(bounds[:-1], bounds[1:]):
        if a >= b:
            return None
        p = _core_pieces(segs[a:b], S)
        if p is None:
            return None
        out.append(p)
    return out


def _envelope(cores, S):
    env = [0] * S
    for pieces in cores:
        for i, (w, _, _) in enumerate(pieces):
            if w > env[i]:
                env[i] = w
    return env


def _cost(cores_all, S):
    env = _envelope(cores_all, S)
    return 64 * S + sum(env), env


def _plan_cuts(segs_co, segs_cl):
    """Choose S and per-branch cuts minimizing 64*S + sum(envelope)."""
    def balanced(segs):
        widths = np.array([w for (_, _, w) in segs])
        csum = np.cumsum(widths)
        cuts = []
        for i in range(1, NQ):
            cuts.append(int(np.argmin(np.abs(csum - i * csum[-1] / NQ))) + 1)
        return tuple(cuts)

    # equal-group-count cuts are always feasible at S = ceil(ngroups / NQ)
    smin = max(-(-len(segs) // NQ) for segs in (segs_co, segs_cl))

    best = None
    for S in range(smin, smin + 7):
        cuts = {}
        cores = {}
        ok = True
        for name, segs in (("co", segs_co), ("cl", segs_cl)):
            c = balanced(segs)
            cs = _branch_cores(segs, c, S)
            if cs is None:
                # widen: fall back to equal group counts
                n = len(segs)
                c = (n // 4, n // 2, 3 * n // 4)
                cs = _branch_cores(segs, c, S)
            if cs is None:
                ok = False
                break
            cuts[name] = c
            cores[name] = cs
        if not ok:
            continue
        for _ in range(3):
            improved = False
            for name, segs in (("co", segs_co), ("cl", segs_cl)):
                other = cores["cl" if name == "co" else "co"]
                c0, c1, c2 = cuts[name]
                bloc = None
                for d0 in range(-3, 4):
                    for d1 in range(-3, 4):
                        for d2 in range(-3, 4):
                            cc = (c0 + d0, c1 + d1, c2 + d2)
                            if not (0 < cc[0] < cc[1] < cc[2] < len(segs)):
                                continue
                            cs = _branch_cores(segs, cc, S)
                            if cs is None:
                                continue
                            cost, _ = _cost(cs + other, S)
                            if bloc is None or cost < bloc[0]:
                                bloc = (cost, cc, cs)
                if bloc is not None and bloc[1] != cuts[name]:
                    cuts[name] = bloc[1]
                    cores[name] = bloc[2]
                    improved = True
            if not improved:
                break
        cost, env = _cost(cores["co"] + cores["cl"], S)
        if best is None or cost < best[0]:
            best = (cost, S, cores["co"] + cores["cl"], env)
    _, S, cores8, env = best
    return S, cores8, env


def _pack_banks(env):
    """Pack envelope widths (desc-sorted) into psum banks with DESCENDING
    capacities, so each later bank's input-DMA -> matmul -> copy -> out-DMA
    chain is shorter and the post-input tail rides on a tiny bank."""
    caps = [BANK, BANK, 288, 144, 72, 36, 18, 9]
    banks = [[]]
    fill = 0
    bi = 0
    for i, w in enumerate(env):
        if banks[bi] and (fill + w > caps[min(bi, len(caps) - 1)]):
            if bi + 1 >= 8:  # psum limit: keep at most 8 banks
                banks[bi].append(i)
                fill += w
                continue
            banks.append([])
            bi += 1
            fill = 0
        banks[bi].append(i)
        fill += w
    return [bk for bk in banks if bk]


def _plan(co_go, cl_go):
    segs_co = _segments(co_go)
    segs_cl = _segments(cl_go)
    S, cores8, env = _plan_cuts(segs_co, segs_cl)
    banks = _pack_banks(env)
    # final slot order: bank-major
    order = [i for bk in banks for i in bk]
    rank_to_slot = {r: s for s, r in enumerate(order)}
    widths = [env[r] for r in order]               # per final slot
    offs = np.concatenate([[0], np.cumsum(widths)]).astype(np.int64)
    CW = int(offs[-1])
    bank_meta = []                                  # (slot_lo, nslots, c_lo, c_hi)
    s = 0
    for bk in banks:
        bank_meta.append((s, len(bk), int(offs[s]), int(offs[s + len(bk)])))
        s += len(bk)
    # per-core slot fill: list over cores of list of (slot, w_real, g, cstart)
    core_fill = []
    for pieces in cores8:
        fill = []
        for r, (w, g, cs) in enumerate(pieces):
            fill.append((rank_to_slot[r], w, g, cs))
        core_fill.append(fill)
    return {
        "S": len(order), "env": tuple(widths), "offs": offs, "CW": CW,
        "banks": bank_meta, "core_fill": core_fill,
    }


# ----------------------------------------------------------------- program

def _program(env, banks, dt=mybir.dt.bfloat16):
    S = len(env)
    offs = np.concatenate([[0], np.cumsum(env)]).astype(np.int64)
    CW = int(offs[-1])
    nc = bacc.Bacc("TRN2", target_bir_lowering=False, debug=False, num_devices=8)
    xm_d = nc.dram_tensor("xm", [128, S, KC * 64], dt, kind="ExternalInput")
    # wt is bank-major flat: bank b occupies cols [KC*c_lo, KC*c_hi) with
    # inner layout [KC, wb] -- keeps every DMA's contiguous run >= 512B
    wt_d = nc.dram_tensor("wt", [128, KC * CW], dt, kind="ExternalInput")
    o_d = nc.dram_tensor("o", [64, CW], dt, kind="ExternalOutput")

    nbk = len(banks)
    with tile.TileContext(nc) as tc:
        with (
            tc.tile_pool(name="xp", bufs=1) as xp,
            tc.tile_pool(name="wp", bufs=1) as wp,
            tc.tile_pool(name="op", bufs=1) as op,
            tc.tile_pool(name="ps", bufs=1, space="PSUM") as ps,
        ):
            xts = []
            wts = []
            for bi, (s_lo, nsl, c_lo, c_hi) in enumerate(banks):
                wb = c_hi - c_lo
                xt = xp.tile([128, nsl, KC * 64], dt, name=f"xt{bi}")
                nc.sync.dma_start(xt[:], xm_d[:, s_lo : s_lo + nsl, :])
                wt = wp.tile([128, KC * wb], dt, name=f"wt{bi}")
                nc.scalar.dma_start(wt[:], wt_d[:, KC * c_lo : KC * c_hi])
                xts.append(xt)
                wts.append(wt)

            for bi, (s_lo, nsl, c_lo, c_hi) in enumerate(banks):
                used = c_hi - c_lo
                xt, wt = xts[bi], wts[bi]
                acc = ps.tile([64, BANK], mybir.dt.float32, name=f"acc{bi}")
                for j in range(nsl):
                    s = s_lo + j
                    f0 = int(offs[s] - c_lo)
                    w = env[s]
                    for k in range(KC):
                        nc.tensor.matmul(
                            acc[0:64, f0 : f0 + w],
                            xt[:, j, k * 64 : (k + 1) * 64],
                            wt[:, k * used + f0 : k * used + f0 + w],
                            start=(k == 0),
                            stop=(k == KC - 1),
                        )
                ob = op.tile([64, used], dt, name=f"ob{bi}")
                half = (used // 2) & ~1
                if half >= 192:
                    nc.vector.tensor_copy(ob[:, 0:half], acc[0:64, 0:half])
                    nc.scalar.copy(ob[:, half:used], acc[0:64, half:used])
                else:
                    nc.vector.tensor_copy(ob[:], acc[0:64, 0:used])
                # keep the tail bank's output on the fast HWDGE path of the
                # by-then-idle sync queue; middle banks alternate elsewhere
                eng = (nc.gpsimd, nc.scalar)[bi % 2] if bi < nbk - 1 else nc.sync
                eng.dma_start(o_d[:, c_lo:c_hi], ob[:])

    nc.compile()
    return nc


# ----------------------------------------------------------------- host glue

def _host_prep(x, W, fill, plan, goff):
    """Build xm/wt for one core.  fill: list of (slot, w_real, g, cstart)."""
    bf = ml_dtypes.bfloat16
    S, offs, CW = plan["S"], plan["offs"], plan["CW"]
    slot_bank = {}
    for s_lo, nsl, c_lo, c_hi in plan["banks"]:
        for s in range(s_lo, s_lo + nsl):
            slot_bank[s] = (c_lo, c_hi)
    xm = np.zeros((128, S, KC * 64), bf)
    wt = np.zeros((128, KC * CW), bf)
    for slot, w, g, cs in fill:
        # x[b, g, h] -> [128 part(h%128), k, b]
        xg = x[:, goff + g, :].reshape(B, KC, 128).transpose(2, 1, 0)
        xm[:, slot, :] = xg.reshape(128, KC * 64).astype(bf)
        # W[c, h] -> [128, k, w] into the bank-major flat layout
        wg = W[cs : cs + w].reshape(w, KC, 128).transpose(2, 1, 0).astype(bf)
        c_lo, c_hi = slot_bank[slot]
        used = c_hi - c_lo
        f0 = int(offs[slot]) - c_lo
        for k in range(KC):
            wt[:, KC * c_lo + k * used + f0 : KC * c_lo + k * used + f0 + w] = wg[:, k, :]
    return {"xm": xm, "wt": wt}


def kernel(x, co_W, cl_W, co_b, cl_b, co_group_of, cl_group_of, co_index,
           cl_index, group_len):
    x = np.asarray(x, np.float32)
    G = int(group_len)
    plan = _plan(co_group_of, cl_group_of)
    S, env, offs, CW = plan["S"], plan["env"], plan["offs"], plan["CW"]

    key = ("v6env", env, tuple(b[:2] for b in plan["banks"]))
    if key not in _cache:
        _cache[key] = _program(list(env), plan["banks"])
    nc = _cache[key]

    Ws = (np.asarray(co_W, np.float32)[0], np.asarray(cl_W, np.float32)[0])
    in_maps = []
    for c in range(8):
        bi = c // NQ
        in_maps.append(_host_prep(x, Ws[bi], plan["core_fill"][c], plan, bi * G))

    res = run_bass_kernel_spmd(nc, in_maps, list(range(8)))

    outs = []
    for bi, bias, index in ((0, co_b, co_index), (1, cl_b, cl_index)):
        full = np.empty((B, NCLS), np.float32)
        for q in range(NQ):
            o = np.asarray(res.results[bi * NQ + q]["o"]).astype(np.float32)
            for slot, w, g, cs in plan["core_fill"][bi * NQ + q]:
                f0 = int(offs[slot])
                full[:, cs : cs + w] = o[:, f0 : f0 + w]
        full += np.asarray(bias, np.float32)
        outs.append(full[:, np.asarray(index).astype(np.int64)])
    return outs[0], outs[1]


# revision 11
# speedup vs baseline: 1.1103x; 1.1103x over previous
"""GroupWiseLinear Trainium2 kernel.

out[b, c] = dot(W[0, c, :], x[b, group_of[c], :]) + bias[0, c], then a final
class-permutation gather, for two independent branches (co / cl).

Sharding: 8 cores = 2 branches x 4 class-shards.  Shard boundaries are chosen
at group boundaries so no group's x is loaded by two cores.  Each core's
ragged class range is cut into "pieces" (one group each, <= 512 classes); the
piece widths of all 8 cores are rank-matched into a single static width
ENVELOPE so every core runs the same instruction stream (SPMD) on different
data:

  - xm: [128, S, 256]    per-slot x^T (one 64KB tile per piece, H-major)
  - wt: [128, 4, CW]     W^T packed to the envelope layout (pad = garbage)
  - o:  [64, CW]         bf16 output, envelope layout (pad ignored on host)

Device work per slot: 4 K-chunk matmuls (x stationary [128,64], W moving
[128, w_env]) accumulating into a PSUM bank shared by several slots; each
bank is then copied (f32->bf16, two engine-parallel halves) to SBUF and
DMA'd out.  Bias and the final class permutation are applied on host.
"""

import heapq

import ml_dtypes
import numpy as np

import concourse.bacc as bacc
import concourse.tile as tile
from concourse import mybir
from concourse.bass_utils import run_bass_kernel_spmd

B = 64          # batch
H = 512         # hidden
NCLS = 4096     # classes per branch
KC = H // 128   # contraction chunks
NQ = 4          # class-shards per branch
BANK = 512      # psum bank width (f32 cols)

_cache = {}


# ----------------------------------------------------------------- planning

def _segments(go):
    """group_of (sorted) -> list of (group, class_start, width)."""
    go = np.asarray(go).astype(np.int64)
    segs = []
    i = 0
    n = len(go)
    while i < n:
        j = i
        while j < n and go[j] == go[i]:
            j += 1
        segs.append((int(go[i]), i, j - i))
        i = j
    return segs


def _core_pieces(segs, S):
    """Split a core's segments into exactly S pieces (halve the largest),
    returning them sorted by descending width.  None if > S segments."""
    if len(segs) > S:
        return None
    h = [(-w, g, cs, w) for (g, cs, w) in segs]
    heapq.heapify(h)
    n = len(h)
    while n < S:
        _, g, cs, w = heapq.heappop(h)
        a = w // 2
        b = w - a
        if a == 0:  # cannot split further; put back and stop
            heapq.heappush(h, (-w, g, cs, w))
            break
        heapq.heappush(h, (-a, g, cs, a))
        heapq.heappush(h, (-b, g, cs + a, b))
        n += 1
    return sorted(((w, g, cs) for (_, g, cs, w) in h), reverse=True)


def _branch_cores(segs, cuts, S):
    """cuts: 3 group-index boundaries -> 4 cores' piece lists."""
    bounds = [0] + list(cuts) + [len(segs)]
    out = []
    for a, b in zip(bounds[:-1], bounds[1:]):
        if a >= b:
            return None
        p = _core_pieces(segs[a:b], S)
        if p is None:
            return None
        out.append(p)
    return out


def _envelope(cores, S):
    env = [0] * S
    for pieces in cores:
        for i, (w, _, _) in enumerate(pieces):
            if w > env[i]:
                env[i] = w
    return env


def _cost(cores_all, S):
    env = _envelope(cores_all, S)
    return 64 * S + sum(env), env


def _plan_cuts(segs_co, segs_cl):
    """Choose S and per-branch cuts minimizing 64*S + sum(envelope)."""
    def balanced(segs):
        widths = np.array([w for (_, _, w) in segs])
        csum = np.cumsum(widths)
        cuts = []
        for i in range(1, NQ):
            cuts.append(int(np.argmin(np.abs(csum - i * csum[-1] / NQ))) + 1)
        return tuple(cuts)

    # equal-group-count cuts are always feasible at S = ceil(ngroups / NQ)
    smin = max(-(-len(segs) // NQ) for segs in (segs_co, segs_cl))

    best = None
    for S in range(smin, smin + 7):
        cuts = {}
        cores = {}
        ok = True
        for name, segs in (("co", segs_co), ("cl", segs_cl)):
            c = balanced(segs)
            cs = _branch_cores(segs, c, S)
            if cs is None:
                # widen: fall back to equal group counts
                n = len(segs)
                c = (n // 4, n // 2, 3 * n // 4)
                cs = _branch_cores(segs, c, S)
            if cs is None:
                ok = False
                break
            cuts[name] = c
            cores[name] = cs
        if not ok:
            continue
        for _ in range(3):
            improved = False
            for name, segs in (("co", segs_co), ("cl", segs_cl)):
                other = cores["cl" if name == "co" else "co"]
                c0, c1, c2 = cuts[name]
                bloc = None
                for d0 in range(-3, 4):
                    for d1 in range(-3, 4):
                        for d2 in range(-3, 4):
                            cc = (c0 + d0, c1 + d1, c2 + d2)
                            if not (0 < cc[0] < cc[1] < cc[2] < len(segs)):
                                continue
                            cs = _branch_cores(segs, cc, S)
                            if cs is None:
                                continue
                            cost, _ = _cost(cs + other, S)
                            if bloc is None or cost < bloc[0]:
                                bloc = (cost, cc, cs)
                if bloc is not None and bloc[1] != cuts[name]:
                    cuts[name] = bloc[1]
                    cores[name] = bloc[2]
                    improved = True
            if not improved:
                break
        cost, env = _cost(cores["co"] + cores["cl"], S)
        if best is None or cost < best[0]:
            best = (cost, S, cores["co"] + cores["cl"], env)
    _, S, cores8, env = best
    return S, cores8, env


def _pack_banks(env):
    """Pack envelope widths (desc-sorted) into psum banks with DESCENDING
    capacities, so each later bank's input-DMA -> matmul -> copy -> out-DMA
    chain is shorter and the post-input tail rides on a tiny bank."""
    remaining = list(range(len(env)))  # env is sorted desc
    rem_sum = sum(env)
    banks = []
    while remaining:
        if len(banks) == 7:  # psum limit: at most 8 banks
            banks.append(remaining)
            break
        target = min(BANK, max(16, -(-rem_sum * 62 // 100)))
        bank = []
        fill = 0
        for i in list(remaining):
            if fill >= target:
                break
            if fill + env[i] <= BANK:
                bank.append(i)
                fill += env[i]
                remaining.remove(i)
        banks.append(bank)
        rem_sum -= fill
    return [bk for bk in banks if bk]


def _plan(co_go, cl_go):
    segs_co = _segments(co_go)
    segs_cl = _segments(cl_go)
    S, cores8, env = _plan_cuts(segs_co, segs_cl)
    banks = _pack_banks(env)
    # final slot order: bank-major
    order = [i for bk in banks for i in bk]
    rank_to_slot = {r: s for s, r in enumerate(order)}
    widths = [env[r] for r in order]               # per final slot
    offs = np.concatenate([[0], np.cumsum(widths)]).astype(np.int64)
    CW = int(offs[-1])
    bank_meta = []                                  # (slot_lo, nslots, c_lo, c_hi)
    s = 0
    for bk in banks:
        bank_meta.append((s, len(bk), int(offs[s]), int(offs[s + len(bk)])))
        s += len(bk)
    # per-core slot fill: list over cores of list of (slot, w_real, g, cstart)
    core_fill = []
    for pieces in cores8:
        fill = []
        for r, (w, g, cs) in enumerate(pieces):
            fill.append((rank_to_slot[r], w, g, cs))
        core_fill.append(fill)
    return {
        "S": len(order), "env": tuple(widths), "offs": offs, "CW": CW,
        "banks": bank_meta, "core_fill": core_fill,
    }


# ----------------------------------------------------------------- program

def _program(env, banks, dt=mybir.dt.bfloat16):
    S = len(env)
    offs = np.concatenate([[0], np.cumsum(env)]).astype(np.int64)
    CW = int(offs[-1])
    nc = bacc.Bacc("TRN2", target_bir_lowering=False, debug=False, num_devices=8)
    xm_d = nc.dram_tensor("xm", [128, S, KC * 64], dt, kind="ExternalInput")
    # wt is bank-major flat: bank b occupies cols [KC*c_lo, KC*c_hi) with
    # inner layout [KC, wb] -- keeps every DMA's contiguous run >= 512B
    wt_d = nc.dram_tensor("wt", [128, KC * CW], dt, kind="ExternalInput")
    o_d = nc.dram_tensor("o", [64, CW], dt, kind="ExternalOutput")

    nbk = len(banks)
    with tile.TileContext(nc) as tc:
        with (
            tc.tile_pool(name="xp", bufs=1) as xp,
            tc.tile_pool(name="wp", bufs=1) as wp,
            tc.tile_pool(name="op", bufs=1) as op,
            tc.tile_pool(name="ps", bufs=1, space="PSUM") as ps,
        ):
            xts = []
            wts = []
            for bi, (s_lo, nsl, c_lo, c_hi) in enumerate(banks):
                wb = c_hi - c_lo
                xt = xp.tile([128, nsl, KC * 64], dt, name=f"xt{bi}")
                nc.sync.dma_start(xt[:], xm_d[:, s_lo : s_lo + nsl, :])
                wt = wp.tile([128, KC * wb], dt, name=f"wt{bi}")
                nc.scalar.dma_start(wt[:], wt_d[:, KC * c_lo : KC * c_hi])
                xts.append(xt)
                wts.append(wt)

            for bi, (s_lo, nsl, c_lo, c_hi) in enumerate(banks):
                used = c_hi - c_lo
                xt, wt = xts[bi], wts[bi]
                acc = ps.tile([64, BANK], mybir.dt.float32, name=f"acc{bi}")
                for j in range(nsl):
                    s = s_lo + j
                    f0 = int(offs[s] - c_lo)
                    w = env[s]
                    for k in range(KC):
                        nc.tensor.matmul(
                            acc[0:64, f0 : f0 + w],
                            xt[:, j, k * 64 : (k + 1) * 64],
                            wt[:, k * used + f0 : k * used + f0 + w],
                            start=(k == 0),
                            stop=(k == KC - 1),
                        )
                ob = op.tile([64, used], dt, name=f"ob{bi}")
                half = (used // 2) & ~1
                if half >= 192:
                    nc.vector.tensor_copy(ob[:, 0:half], acc[0:64, 0:half])
                    nc.scalar.copy(ob[:, half:used], acc[0:64, half:used])
                else:
                    nc.vector.tensor_copy(ob[:], acc[0:64, 0:used])
                # keep the tail bank's output on the fast HWDGE path of the
                # by-then-idle sync queue; middle banks alternate elsewhere
                eng = (nc.gpsimd, nc.scalar)[bi % 2] if bi < nbk - 1 else nc.sync
                eng.dma_start(o_d[:, c_lo:c_hi], ob[:])

    nc.compile()
    return nc


# ----------------------------------------------------------------- host glue

def _host_prep(x, W, fill, plan, goff):
    """Build xm/wt for one core.  fill: list of (slot, w_real, g, cstart)."""
    bf = ml_dtypes.bfloat16
    S, offs, CW = plan["S"], plan["offs"], plan["CW"]
    slot_bank = {}
    for s_lo, nsl, c_lo, c_hi in plan["banks"]:
        for s in range(s_lo, s_lo + nsl):
            slot_bank[s] = (c_lo, c_hi)
    xm = np.zeros((128, S, KC * 64), bf)
    wt = np.zeros((128, KC * CW), bf)
    for slot, w, g, cs in fill:
        # x[b, g, h] -> [128 part(h%128), k, b]
        xg = x[:, goff + g, :].reshape(B, KC, 128).transpose(2, 1, 0)
        xm[:, slot, :] = xg.reshape(128, KC * 64).astype(bf)
        # W[c, h] -> [128, k, w] into the bank-major flat layout
        wg = W[cs : cs + w].reshape(w, KC, 128).transpose(2, 1, 0).astype(bf)
        c_lo, c_hi = slot_bank[slot]
        used = c_hi - c_lo
        f0 = int(offs[slot]) - c_lo
        for k in range(KC):
            wt[:, KC * c_lo + k * used + f0 : KC * c_lo + k * used + f0 + w] = wg[:, k, :]
    return {"xm": xm, "wt": wt}


def kernel(x, co_W, cl_W, co_b, cl_b, co_group_of, cl_group_of, co_index,
           cl_index, group_len):
    x = np.asarray(x, np.float32)
    G = int(group_len)
    plan = _plan(co_group_of, cl_group_of)
    S, env, offs, CW = plan["S"], plan["env"], plan["offs"], plan["CW"]

    key = ("v6env", env, tuple(b[:2] for b in plan["banks"]))
    if key not in _cache:
        _cache[key] = _program(list(env), plan["banks"])
    nc = _cache[key]

    Ws = (np.asarray(co_W, np.float32)[0], np.asarray(cl_W, np.float32)[0])
    in_maps = []
    for c in range(8):
        bi = c // NQ
        in_maps.append(_host_prep(x, Ws[bi], plan["core_fill"][c], plan, bi * G))

    res = run_bass_kernel_spmd(nc, in_maps, list(range(8)))

    outs = []
    for bi, bias, index in ((0, co_b, co_index), (1, cl_b, cl_index)):
        full = np.empty((B, NCLS), np.float32)
        for q in range(NQ):
            o = np.asarray(res.results[bi * NQ + q]["o"]).astype(np.float32)
            for slot, w, g, cs in plan["core_fill"][bi * NQ + q]:
                f0 = int(offs[slot])
                full[:, cs : cs + w] = o[:, f0 : f0 + w]
        full += np.asarray(bias, np.float32)
        outs.append(full[:, np.asarray(index).astype(np.int64)])
    return outs[0], outs[1]
